# revision 1
# baseline (speedup 1.0000x reference)
"""Trainium2 Bass kernel for nn_ControlModel_g (phi^4 lattice control-variate loss).

Math reformulation (validated to fp32 accuracy against the jax reference):

  The reference evaluates, for each of 16 signed lattice symmetries t and all
  V=256 torus translations s, the tiny MLP g (256->128->1) on the transformed+
  shifted configs, plus its input-gradient at site (0,0), combined with the
  phi^4 force into F[b]; loss = mean((computeO(x) - F - muO)^2).

  1. Symmetry transforms move from x onto W1 (g(T_{-s} R x) = g_R(T_{-s'} x)
     with spatially-transformed weights), so all shifted inputs derive from x
     alone and the force/gradient corrections become fixed permutations.
  2. With b1 == 0 (always true for this model), tanh oddness makes the 8
     sign=-1 transforms algebraically redundant -> half the compute.
  3. The column translation j folds into 16 rotated weight copies
     (W1JBIG[(a,c), (j,r,h)] = W1_r[a, (c-j)%16, h]); the row translation i
     folds into a small shifted-x matrix SH2[(a,c), (i,b)] = x[b,(a+i)%16,c].
     The device work is then one dense matmul Z = SH2^T @ W1JBIG
     (512 x 16384), tanh, and two h-weighted reductions:
         GV = sum_h W2[h] * tanh(Z),   GD = sum_h (W2*W1[0])[h] * tanh(Z)^2
  4. Sharding: data-parallel over the j columns - core k takes j in {2k,2k+1}
     (2048 of the 16384 output columns). No collectives needed; the final
     O(B*V) combine (force permutations, computeO, loss) is host-side numpy.

Device schedule: bf16 inputs chunked per weight-set across two DMA
queues; all Z matmuls form one continuous PE streak; the GV/GD reductions run
as 128 small matmuls with tanh/tanh^2 chunks as the stationary operand and
the (hi,lo)-split reduce weights as a 2-column moving operand, so all results
land contiguously in one [128, 256] PSUM accumulator -> one small output DMA.
"""

import numpy as np
import ml_dtypes

L = 16
Y = 4
KAPPA = 0.25
LAM = 0.5
B = 32
V = L * L          # 256
H = 128
NCORES = 8
JPER = L // NCORES         # j values per core = 2
NCOLS = JPER * 8 * H       # 2048 columns per core (j, r, h)
M = L * B                  # 512 rows (i, b)
NSET = 4                   # weight chunks of 512 columns
NGRP = JPER * 8            # 16 reduce groups per core
WARM_MMS = 55              # PE p-state warm-up matmuls

# ---------------------------------------------------------------------------
# host-side lattice helpers
# ---------------------------------------------------------------------------

def _force(phi):
    nbr = (np.roll(phi, 1, 1) + np.roll(phi, -1, 1)
           + np.roll(phi, 1, 2) + np.roll(phi, -1, 2))
    return 2.0 * KAPPA * nbr - 2.0 * phi - 4.0 * LAM * phi * (phi * phi - 1.0)


def _computeO(x):
    x0 = x.mean(axis=1)
    x0 = x0 - x0.mean(axis=0, keepdims=True)
    return (x0 * np.roll(x0, -Y, axis=1)).mean(axis=1)


def _spatial_ops():
    ops = []
    for k in range(4):
        ops.append(lambda y, k=k: np.rot90(y, k=k, axes=(0, 1)))
        ops.append(lambda y, k=k: np.flip(np.rot90(y, k=k, axes=(0, 1)), axis=0))
    return ops


def _op_tables():
    """Per spatial op r: inverse site permutation (for W1) and the force
    permutation mu_r[s] = pi_r(rho_r^{-1}(s))."""
    ops = _spatial_ops()
    IDX = np.arange(V).reshape(L, L)
    inv_perms, mus = [], []
    for op in ops:
        pi = op(IDX).reshape(-1)
        inv = np.empty(V, np.int64)
        inv[pi] = np.arange(V)
        inv_perms.append(inv)
        rho = np.empty(V, np.int64)
        opIDX = op(IDX)
        for i in range(L):
            for j in range(L):
                shifted = np.roll(np.roll(opIDX, -i, 0), -j, 1).reshape(V)
                rho[i * L + j] = shifted[inv][0]
        rho_inv = np.empty(V, np.int64)
        rho_inv[rho] = np.arange(V)
        mus.append(pi[rho_inv])
    return inv_perms, mus


_TABLES = None

def _tables():
    global _TABLES
    if _TABLES is None:
        _TABLES = _op_tables()
    return _TABLES


# ---------------------------------------------------------------------------
# device program (built once, cached)
# ---------------------------------------------------------------------------

_PROG = None

def _build_program(reps=None, dma_in_loop=False, out_in_loop=True):
    import concourse.bass as bass
    import concourse.tile as tile
    from concourse import bacc, mybir

    f32 = mybir.dt.float32
    bf16 = mybir.dt.bfloat16
    MUL = mybir.AluOpType.mult
    TANH = mybir.ActivationFunctionType.Tanh

    nc = bacc.Bacc("TRN2", target_bir_lowering=False, debug=False,
                   num_devices=NCORES)
    # head[k, p, (i b | s0 cols)] packs SH2 with weight-set 0 so one DMA per
    # k-tile unblocks the first Z matmuls; w1r holds sets 1..3.
    # Group g = 4s + gl = jl*8 + r; rw columns (W2hi, W2lo, CWhi, CWlo).
    head_d = nc.dram_tensor("head", (2, 128, M + 512), bf16,
                            kind="ExternalInput")
    w1r_d = nc.dram_tensor("w1r", (NSET - 1, 2, 128, 512), bf16,
                           kind="ExternalInput")
    rw_d = nc.dram_tensor("rw", (128, 4), bf16, kind="ExternalInput")
    out_d = nc.dram_tensor("gvgd", (128, 4 * 64), f32, kind="ExternalOutput")

    with tile.TileContext(nc) as tc:
        with (
            tc.tile_pool(name="consts", bufs=2 if dma_in_loop else 1) as cpool,
            tc.tile_pool(name="zp", bufs=3, space=bass.MemorySpace.PSUM) as zpool,
            tc.tile_pool(name="rp", bufs=2 if dma_in_loop else 1,
                         space=bass.MemorySpace.PSUM) as rpool,
            tc.tile_pool(name="work", bufs=2 if dma_in_loop else 1) as wpool,
        ):
            # Prime the Act engine's tanh table while DMAs are in flight.
            zt0 = wpool.tile([128, 1], f32, tag="prime_in")
            pr0 = wpool.tile([128, 1], bf16, tag="prime_out")
            nc.gpsimd.memset(zt0[:], 0.0)
            nc.scalar.activation(pr0[:], zt0[:], TANH)
            # Warm the PE p-state with dependency-free dummy matmuls so the
            # tensor engine is at full clock when the first real weights land
            # (~3.6us): without this the first sets run at half rate.
            warm_src = wpool.tile([128, 64], bf16, tag="warm_src")
            nc.gpsimd.memset(warm_src[:], 0.03125)

            def load_consts():
                # SP queue: head k0, w1 s1k0, s2k0, s3k0, rw (then out later)
                # Act queue (drains before the first tanh is ready): head k1,
                # w1 s1k1, s2k1, s3k1
                head_t = [cpool.tile([128, M + 512], bf16, tag=f"head{k}",
                                     name=f"head{k}") for k in range(2)]
                w1r_t = [[cpool.tile([128, 512], bf16, tag=f"w1_{s}_{k}",
                                     name=f"w1_{s}_{k}")
                          for k in range(2)] for s in range(1, NSET)]
                rw_t = cpool.tile([128, 4], bf16, tag="rw")
                nc.sync.dma_start(head_t[0][:], head_d[0])
                nc.scalar.dma_start(head_t[1][:], head_d[1])
                nc.sync.dma_start(w1r_t[0][0][:], w1r_d[0, 0])
                nc.sync.dma_start(w1r_t[0][1][:], w1r_d[0, 1])
                for s in range(2, NSET):
                    nc.sync.dma_start(w1r_t[s - 1][0][:], w1r_d[s - 1, 0])
                    nc.scalar.dma_start(w1r_t[s - 1][1][:], w1r_d[s - 1, 1])
                nc.sync.dma_start(rw_t[:], rw_d[:])
                sh_t = [head_t[k][:, 0:M] for k in range(2)]
                w1_t = [[head_t[k][:, M:M + 512] for k in range(2)]] + \
                       [[w1r_t[s - 1][k][:] for k in range(2)]
                        for s in range(1, NSET)]
                return sh_t, w1_t, rw_t

            if not dma_in_loop:
                consts = load_consts()

            def body():
                sh_t, w1_t, rw_t = load_consts() if dma_in_loop else consts
                # split accumulators/staging so early-set results don't pick
                # up tile-granular dependencies on set 3's reduce matmuls
                racc0 = rpool.tile([128, 192], f32, tag="racc0")
                racc1 = rpool.tile([128, 64], f32, tag="racc1")
                rsb0 = wpool.tile([128, 192], f32, tag="rsb0")
                rsb1 = wpool.tile([128, 64], f32, tag="rsb1")
                # PE p-state warm-up: dependency-free matmuls into a zt-pool
                # buffer (later zt matmuls overwrite it with start=True)
                warm_zt = zpool.tile([128, 1024], f32, tag="zt", name="warm_zt")
                for _ in range(WARM_MMS):
                    nc.tensor.matmul(warm_zt[0:64, 0:64], warm_src[:, 0:64],
                                     warm_src[:], start=True, stop=True)

                ats, asqs = {}, {}

                def emit_z_tanh(p):
                    s, zp = divmod(p, 2)
                    zt = zpool.tile([128, 1024], f32, tag="zt", name="zt")
                    for half in range(2):
                        g128 = (zp * 2 + half) * 128
                        hs = slice(half * 512, (half + 1) * 512)
                        for k in range(2):
                            nc.tensor.matmul(
                                zt[:, hs],
                                w1_t[s][k][:, g128:g128 + 128],
                                sh_t[k],
                                start=(k == 0),
                                stop=(k == 1),
                            )
                    at = wpool.tile([128, 1024], bf16, tag=f"at{p}", name="at")
                    nc.scalar.activation(at[:], zt[:], TANH)
                    ats[p] = at

                def emit_asq(p):
                    asq = wpool.tile([128, 1024], bf16, tag=f"asq{p}",
                                     name="asq")
                    nc.vector.tensor_tensor(asq[:], ats[p][:], ats[p][:], MUL)
                    asqs[p] = asq

                def emit_red(p, which):
                    s, zp = divmod(p, 2)
                    racc, roff = (racc0, s * 64) if s < NSET - 1 else (racc1, 0)
                    src = ats[p] if which == 0 else asqs[p]
                    for c8 in range(8):
                        off = roff + (zp * 8 + c8) * 4 + 2 * which
                        cs = slice(c8 * 128, (c8 + 1) * 128)
                        nc.tensor.matmul(racc[:, off:off + 2], src[:, cs],
                                         rw_t[:, 2 * which:2 * which + 2],
                                         start=True, stop=True)

                NP = 2 * NSET
                # software-pipelined emission: PE never waits on Act/DVE
                for p in range(NP):
                    emit_z_tanh(p)
                    emit_asq(p)
                    if p >= 2:
                        emit_red(p - 2, 0)
                    if p >= 3:
                        emit_red(p - 3, 1)
                emit_red(NP - 2, 0)
                emit_red(NP - 3, 1)
                # sets 0..2 complete once gd(NP-3) lands: ship them early
                nc.vector.tensor_copy(rsb0[:], racc0[:])
                if out_in_loop:
                    nc.sync.dma_start(out_d[:, 0:192], rsb0[:])
                emit_red(NP - 1, 0)
                emit_red(NP - 2, 1)
                emit_red(NP - 1, 1)
                nc.vector.tensor_copy(rsb1[:], racc1[:])
                if out_in_loop:
                    nc.sync.dma_start(out_d[:, 192:256], rsb1[:])
                return rsb1

            if reps is None:
                body()
            else:
                with tc.For_i(0, reps, 1):
                    rsb = body()
                if not out_in_loop:
                    nc.sync.dma_start(out_d[:, 192:256], rsb[:])

    nc.compile()
    return nc


def _get_program():
    global _PROG
    if _PROG is None:
        _PROG = _build_program()
    return _PROG


# ---------------------------------------------------------------------------
# numpy fallback (general b1; never hit for this model's inputs)
# ---------------------------------------------------------------------------

def _numpy_reference(x, W1, b1, W2, b2, muO):
    def transforms(x):
        outs = []
        for sign in (1.0, -1.0):
            sx = sign * x
            for k in range(4):
                rx = np.rot90(sx, k=k, axes=(1, 2))
                outs.append(rx)
                outs.append(np.flip(rx, axis=1))
        return np.stack(outs)

    idx = (np.arange(L)[:, None] + np.arange(L)[None, :]) % L
    Ftot = np.zeros(B, np.float32)
    for tx in transforms(x):
        fx = _force(tx).reshape(B, V)
        sh = tx[:, idx, :][:, :, :, idx]
        shifts = np.transpose(sh, (1, 3, 0, 2, 4)).reshape(V, B, V)
        z = shifts @ W1 + b1
        h = np.tanh(z)
        gvals = h @ W2 + b2[0]
        grads = ((1.0 - h * h) * W2) @ W1[0]
        Ftot += (grads + gvals * fx.T).sum(axis=0)
    F = Ftot / 16.0
    delta = _computeO(x) - F
    return np.float32(((delta - muO[0]) ** 2).mean())


# ---------------------------------------------------------------------------
# host-side input prep / output decode
# ---------------------------------------------------------------------------

def _prepare_inputs(x, W1, W2):
    inv_perms, _ = _tables()
    W1flat = W1.reshape(V, H)

    # SH2[(a,c), (i,b)] = x[b, (a+i)%L, c]
    SH2 = np.empty((V, M), np.float32)
    for i in range(L):
        SH2[:, i * B:(i + 1) * B] = np.roll(x, -i, axis=1).reshape(B, V).T
    sh2_in = SH2.reshape(2, 128, M).astype(ml_dtypes.bfloat16)

    # W1JBIG columns (jl, r, h); per-core slice j in {2k, 2k+1}
    W1r_imgs = [W1flat[inv].reshape(L, L, H) for inv in inv_perms]
    w1_cores = []
    for k in range(NCORES):
        blk = np.empty((V, JPER, 8, H), np.float32)
        for jl in range(JPER):
            j = JPER * k + jl
            for r in range(8):
                blk[:, jl, r, :] = np.roll(W1r_imgs[r], j, axis=1).reshape(V, H)
        # (2, 128, 2048) -> (set, k, 128, 512)
        w1 = blk.reshape(2, 128, NCOLS).reshape(2, 128, NSET, 512)
        w1_cores.append(np.ascontiguousarray(
            w1.transpose(2, 0, 1, 3)).astype(ml_dtypes.bfloat16))
    # pack [sh2 | set0] per k-tile; ship sets 1..3 separately
    head_cores = [np.concatenate([sh2_in, w1_cores[k][0]], axis=2)
                  for k in range(NCORES)]

    CW = (W1flat[0] * W2).astype(np.float32)

    def _hilo(w):
        hi = w.astype(ml_dtypes.bfloat16)
        lo = (w - hi.astype(np.float32)).astype(ml_dtypes.bfloat16)
        return hi, lo

    rw_in = np.zeros((128, 4), ml_dtypes.bfloat16)
    rw_in[:, 0], rw_in[:, 1] = _hilo(W2)
    rw_in[:, 2], rw_in[:, 3] = _hilo(CW)
    return head_cores, w1_cores, rw_in, CW


def _decode_outputs(results):
    """Per-core (128, 256) f32 -> GV, GD [(i, b, j, r)]."""
    GV = np.empty((L, B, L, 8), np.float32)
    GD = np.empty((L, B, L, 8), np.float32)
    for k in range(NCORES):
        arr = np.asarray(results[k]["gvgd"])        # (128, 256)
        # p = ilo*32 + b ; col = s*64 + (gl*4 + ihi)*4 + q
        a6 = arr.reshape(4, 32, 4, 4, 4, 4)          # [ilo, b, s, gl, ihi, q]
        gv = a6[..., 0] + a6[..., 1]                 # [ilo, b, s, gl, ihi]
        gd = a6[..., 2] + a6[..., 3]
        # i = ihi*4 + ilo ; g = s*4 + gl ; jl = g//8 ; r = g%8
        gv = gv.transpose(4, 0, 1, 2, 3).reshape(L, B, NGRP)  # [i, b, g]
        gd = gd.transpose(4, 0, 1, 2, 3).reshape(L, B, NGRP)
        gv = gv.reshape(L, B, JPER, 8)               # [i, b, jl, r]
        gd = gd.reshape(L, B, JPER, 8)
        GV[:, :, JPER * k:JPER * (k + 1), :] = gv
        GD[:, :, JPER * k:JPER * (k + 1), :] = gd
    return GV, GD


def _combine(x, GV, GD, CW, muO):
    _, mus = _tables()
    fxo = _force(x).reshape(B, V)
    Csum = float(CW.sum())
    Ftot = np.zeros(B, np.float64)
    for r in range(8):
        gval = GV[:, :, :, r].transpose(0, 2, 1).reshape(V, B)
        gdot = Csum - GD[:, :, :, r].transpose(0, 2, 1).reshape(V, B)
        fxt = fxo[:, mus[r]].T
        Ftot += (gdot + gval * fxt).sum(axis=0)
    F = (Ftot / 8.0).astype(np.float32)
    delta = _computeO(x) - F
    return np.float32(((delta - muO[0]) ** 2).mean())


# ---------------------------------------------------------------------------
# entry point
# ---------------------------------------------------------------------------

def kernel(x, W1, b1, W2, b2, muO):
    x = np.asarray(x, np.float32)
    W1 = np.asarray(W1, np.float32)
    b1 = np.asarray(b1, np.float32)
    W2 = np.asarray(W2, np.float32)
    b2 = np.asarray(b2, np.float32)
    muO = np.asarray(muO, np.float32)

    if np.any(b1 != 0.0):
        return _numpy_reference(x, W1, b1, W2, b2, muO)

    head_cores, w1_cores, rw_in, CW = _prepare_inputs(x, W1, W2)

    nc = _get_program()
    from concourse import bass_utils
    in_maps = [{"head": head_cores[k], "w1r": w1_cores[k][1:], "rw": rw_in}
               for k in range(NCORES)]
    res = bass_utils.run_bass_kernel_spmd(nc, in_maps,
                                          core_ids=list(range(NCORES)))

    GV, GD = _decode_outputs(res.results)
    return _combine(x, GV, GD, CW, muO)



# revision 7
# speedup vs baseline: 1.1152x; 1.1152x over previous
"""Trainium2 Bass kernel for nn_ControlModel_g (phi^4 lattice control-variate loss).

Math reformulation (validated to fp32 accuracy against the jax reference):

  The reference evaluates, for each of 16 signed lattice symmetries t and all
  V=256 torus translations s, the tiny MLP g (256->128->1) on the transformed+
  shifted configs, plus its input-gradient at site (0,0), combined with the
  phi^4 force into F[b]; loss = mean((computeO(x) - F - muO)^2).

  1. Symmetry transforms move from x onto W1 (g(T_{-s} R x) = g_R(T_{-s'} x)
     with spatially-transformed weights), so all shifted inputs derive from x
     alone and the force/gradient corrections become fixed permutations.
  2. With b1 == 0 (always true for this model), tanh oddness makes the 8
     sign=-1 transforms algebraically redundant -> half the compute.
  3. The column translation j folds into 16 rotated weight copies
     (W1JBIG[(a,c), (j,r,h)] = W1_r[a, (c-j)%16, h]); the row translation i
     folds into a small shifted-x matrix SH2[(a,c), (i,b)] = x[b,(a+i)%16,c].
     The device work is then one dense matmul Z = SH2^T @ W1JBIG
     (512 x 16384), tanh, and two h-weighted reductions:
         GV = sum_h W2[h] * tanh(Z),   GD = sum_h (W2*W1[0])[h] * tanh(Z)^2
  4. Sharding: data-parallel over the j columns - core k takes j in {2k,2k+1}
     (2048 of the 16384 output columns). No collectives needed; the final
     O(B*V) combine (force permutations, computeO, loss) is host-side numpy.

Device schedule (v2): inputs land as fp8e4; x is split hi+lo so the only fp8
quantization error left is W1's (loss rel err ~7e-3, tol 2e-2). Each of the
16 per-core column chunks is one pair of DoubleRow matmuls (K=256 folded as
[128, 2]) into PSUM; the Activation engine (the bottleneck at 0.83ns/col)
consumes the chunks as 7 variable-size tanh tiles sized to start early and
finish with a small last tile; DVE squares each tile; the h-reductions run as
free small PE matmuls into two PSUM accumulators shipped by two output DMAs
(first 13 chunks early, last 3 at the end).
"""

import numpy as np
import ml_dtypes

L = 16
Y = 4
KAPPA = 0.25
LAM = 0.5
B = 32
V = L * L          # 256
H = 128
NCORES = 8
JPER = L // NCORES         # j values per core = 2
M = L * B                  # 512 rows (i, b)
NG = 16                    # column chunks per core (g = jl*8 + r)
WARM_MMS = 57              # PE p-state warm-up matmuls

# act tile plan: (slot, n_chunks); slots A/B are [128, 1536] f32 PSUM (3
# banks each); accumulators racc0 (chunks 0..12) / racc1 (13..15) take the
# last 2 banks.
TILES = [(0, 1), (1, 3), (0, 3), (1, 3), (0, 3), (1, 2), (0, 1)]
NG0 = 7                    # chunks in racc0 / first output DMA

# ---------------------------------------------------------------------------
# host-side lattice helpers
# ---------------------------------------------------------------------------

def _force(phi):
    nbr = (np.roll(phi, 1, 1) + np.roll(phi, -1, 1)
           + np.roll(phi, 1, 2) + np.roll(phi, -1, 2))
    return 2.0 * KAPPA * nbr - 2.0 * phi - 4.0 * LAM * phi * (phi * phi - 1.0)


def _computeO(x):
    x0 = x.mean(axis=1)
    x0 = x0 - x0.mean(axis=0, keepdims=True)
    return (x0 * np.roll(x0, -Y, axis=1)).mean(axis=1)


def _spatial_ops():
    ops = []
    for k in range(4):
        ops.append(lambda y, k=k: np.rot90(y, k=k, axes=(0, 1)))
        ops.append(lambda y, k=k: np.flip(np.rot90(y, k=k, axes=(0, 1)), axis=0))
    return ops


def _op_tables():
    """Per spatial op r: inverse site permutation (for W1) and the force
    permutation mu_r[s] = pi_r(rho_r^{-1}(s))."""
    ops = _spatial_ops()
    IDX = np.arange(V).reshape(L, L)
    inv_perms, mus = [], []
    for op in ops:
        pi = op(IDX).reshape(-1)
        inv = np.empty(V, np.int64)
        inv[pi] = np.arange(V)
        inv_perms.append(inv)
        rho = np.empty(V, np.int64)
        opIDX = op(IDX)
        for i in range(L):
            for j in range(L):
                shifted = np.roll(np.roll(opIDX, -i, 0), -j, 1).reshape(V)
                rho[i * L + j] = shifted[inv][0]
        rho_inv = np.empty(V, np.int64)
        rho_inv[rho] = np.arange(V)
        mus.append(pi[rho_inv])
    return inv_perms, mus


_TABLES = None

def _tables():
    global _TABLES
    if _TABLES is None:
        _TABLES = _op_tables()
    return _TABLES


# ---------------------------------------------------------------------------
# device program (built once, cached)
# ---------------------------------------------------------------------------

_PROG = None

def _build_program():
    import concourse.bass as bass
    import concourse.tile as tile
    from concourse import bacc, mybir

    f32 = mybir.dt.float32
    bf16 = mybir.dt.bfloat16
    fp8 = mybir.dt.float8e4
    MUL = mybir.AluOpType.mult
    TANH = mybir.ActivationFunctionType.Tanh
    DR = mybir.MatmulPerfMode.DoubleRow

    nc = bacc.Bacc("TRN2", target_bir_lowering=False, debug=False,
                   num_devices=NCORES)
    # shp[p, 0:2, m] = SH2hi[kt*128+p, m]; [2:4] = SH2lo (x hi/lo fp8 pair)
    shp_d = nc.dram_tensor("shp", (128, 4, M), fp8, kind="ExternalInput")
    # w1p[p, g, kt, h] = W1JBIG_core[kt*128+p, g*128+h]
    w1p_d = nc.dram_tensor("w1p", (128, NG, 2, 128), fp8, kind="ExternalInput")
    rw_d = nc.dram_tensor("rw", (128, 4), bf16, kind="ExternalInput")
    out0_d = nc.dram_tensor("gvgd0", (128, NG0 * 16), f32, kind="ExternalOutput")
    out1_d = nc.dram_tensor("gvgd1", (128, (NG - NG0) * 16), f32,
                            kind="ExternalOutput")

    with tile.TileContext(nc) as tc:
        with (
            tc.tile_pool(name="consts", bufs=1) as cpool,
            tc.tile_pool(name="zp", bufs=1, space=bass.MemorySpace.PSUM) as zpool,
            tc.tile_pool(name="rp", bufs=1, space=bass.MemorySpace.PSUM) as rpool,
            tc.tile_pool(name="work", bufs=1) as wpool,
        ):
            # Warm the PE p-state with dependency-free dummy matmuls so the
            # tensor engine is ramped when the first real weights land.
            warm_src = wpool.tile([128, 64], bf16, tag="warm_src")
            nc.gpsimd.memset(warm_src[:], 0.03125)
            # Prime the Act engine's tanh table while DMAs are in flight.
            zt0 = wpool.tile([128, 1], f32, tag="prime_in")
            pr0 = wpool.tile([128, 1], bf16, tag="prime_out")
            nc.gpsimd.memset(zt0[:], 0.0)
            nc.scalar.activation(pr0[:], zt0[:], TANH)

            shp_t = cpool.tile([128, 4, M], fp8, tag="shp")
            w1p_t = cpool.tile([128, NG, 2, 128], fp8, tag="w1p")
            rw_t = cpool.tile([128, 4], bf16, tag="rw")
            nc.sync.dma_start(shp_t[:], shp_d[:])
            nc.scalar.dma_start(w1p_t[:, 0:4], w1p_d[:, 0:4])
            nc.sync.dma_start(w1p_t[:, 4:NG], w1p_d[:, 4:NG])
            nc.scalar.dma_start(rw_t[:], rw_d[:])
            shhi = shp_t[:, 0:2, :]
            shlo = shp_t[:, 2:4, :]

            slots = [zpool.tile([128, 1536], f32, tag="ztA", name="ztA"),
                     zpool.tile([128, 1536], f32, tag="ztB", name="ztB")]
            racc0 = rpool.tile([128, NG0 * 16], f32, tag="racc0")
            racc1 = rpool.tile([128, (NG - NG0) * 16], f32, tag="racc1")
            rsb0 = wpool.tile([128, NG0 * 16], f32, tag="rsb0")
            rsb1 = wpool.tile([128, (NG - NG0) * 16], f32, tag="rsb1")

            for _ in range(WARM_MMS):
                nc.tensor.matmul(slots[0][0:64, 0:64], warm_src[:, 0:64],
                                 warm_src[:], start=True, stop=True)

            # tile t covers chunks [gbase[t], gbase[t]+ng)
            gbase = []
            acc = 0
            for _, ngc in TILES:
                gbase.append(acc)
                acc += ngc
            ats, asqs = {}, {}

            def emit_z(t):
                slot, ngc = TILES[t]
                zt = slots[slot]
                for gi in range(ngc):
                    g = gbase[t] + gi
                    out = zt[:, gi * 512:(gi + 1) * 512]
                    nc.tensor.matmul(out, w1p_t[:, g], shhi,
                                     start=True, stop=False, perf_mode=DR)
                    nc.tensor.matmul(out, w1p_t[:, g], shlo,
                                     start=False, stop=True, perf_mode=DR)

            def emit_act(t):
                slot, ngc = TILES[t]
                at = wpool.tile([128, ngc * 512], bf16, tag=f"at{t}", name="at")
                nc.scalar.activation(at[:], slots[slot][:, 0:ngc * 512], TANH)
                ats[t] = at

            def emit_asq(t):
                _, ngc = TILES[t]
                asq = wpool.tile([128, ngc * 512], bf16, tag=f"asq{t}",
                                 name="asq")
                nc.vector.tensor_tensor(asq[:], ats[t][:], ats[t][:], MUL)
                asqs[t] = asq

            def emit_red(t):
                _, ngc = TILES[t]
                for gi in range(ngc):
                    g = gbase[t] + gi
                    racc, base = (racc0, g * 16) if g < NG0 else \
                                 (racc1, (g - NG0) * 16)
                    for mb in range(4):
                        cs = slice(gi * 512 + mb * 128, gi * 512 + mb * 128 + 128)
                        for which, src in ((0, ats[t]), (1, asqs[t])):
                            off = base + mb * 4 + 2 * which
                            nc.tensor.matmul(racc[:, off:off + 2], src[:, cs],
                                             rw_t[:, 2 * which:2 * which + 2],
                                             start=True, stop=True)

            NT = len(TILES)
            for t in range(NT):
                emit_z(t)
                emit_act(t)
                if t == 4:
                    # racc0 (tiles 0-2) is complete after red(2); emitting the
                    # copy here slots it into the DVE idle window before
                    # asq(4), so the early output DMA departs mid-stream.
                    emit_red(2)
                    nc.vector.tensor_copy(rsb0[:], racc0[:])
                    nc.sync.dma_start(out0_d[:], rsb0[:])
                emit_asq(t)
                if t >= 2 and t != 4:
                    emit_red(t - 2)
            emit_red(NT - 2)
            emit_red(NT - 1)
            nc.vector.tensor_copy(rsb1[:], racc1[:])
            nc.sync.dma_start(out1_d[:], rsb1[:])

    nc.compile()
    return nc


def _get_program():
    global _PROG
    if _PROG is None:
        _PROG = _build_program()
    return _PROG


# ---------------------------------------------------------------------------
# numpy fallback (general b1/b2; never hit for this model's inputs)
# ---------------------------------------------------------------------------

def _numpy_reference(x, W1, b1, W2, b2, muO):
    def transforms(x):
        outs = []
        for sign in (1.0, -1.0):
            sx = sign * x
            for k in range(4):
                rx = np.rot90(sx, k=k, axes=(1, 2))
                outs.append(rx)
                outs.append(np.flip(rx, axis=1))
        return np.stack(outs)

    idx = (np.arange(L)[:, None] + np.arange(L)[None, :]) % L
    Ftot = np.zeros(B, np.float32)
    for tx in transforms(x):
        fx = _force(tx).reshape(B, V)
        sh = tx[:, idx, :][:, :, :, idx]
        shifts = np.transpose(sh, (1, 3, 0, 2, 4)).reshape(V, B, V)
        z = shifts @ W1 + b1
        h = np.tanh(z)
        gvals = h @ W2 + b2[0]
        grads = ((1.0 - h * h) * W2) @ W1[0]
        Ftot += (grads + gvals * fx.T).sum(axis=0)
    F = Ftot / 16.0
    delta = _computeO(x) - F
    return np.float32(((delta - muO[0]) ** 2).mean())


# ---------------------------------------------------------------------------
# host-side input prep / output decode
# ---------------------------------------------------------------------------

def _prepare_inputs(x, W1, W2):
    inv_perms, _ = _tables()
    W1flat = W1.reshape(V, H)
    e4 = ml_dtypes.float8_e4m3

    # SH2[(a,c), (i,b)] = x[b, (a+i)%L, c]; x split hi+lo in fp8
    SH2 = np.empty((V, M), np.float32)
    for i in range(L):
        SH2[:, i * B:(i + 1) * B] = np.roll(x, -i, axis=1).reshape(B, V).T
    shhi = SH2.astype(e4)
    shlo = (SH2 - shhi.astype(np.float32)).astype(e4)
    shp = np.empty((128, 4, M), e4)
    for kt in range(2):
        shp[:, kt] = shhi[kt * 128:(kt + 1) * 128]
        shp[:, 2 + kt] = shlo[kt * 128:(kt + 1) * 128]

    # W1JBIG columns (jl, r, h); per-core slice j in {2k, 2k+1}
    W1r_imgs = [W1flat[inv].reshape(L, L, H) for inv in inv_perms]
    w1_cores = []
    for k in range(NCORES):
        blk = np.empty((V, JPER, 8, H), np.float32)
        for jl in range(JPER):
            j = JPER * k + jl
            for r in range(8):
                blk[:, jl, r, :] = np.roll(W1r_imgs[r], j, axis=1).reshape(V, H)
        w1c = blk.reshape(V, NG, 128).astype(e4)   # [kt*128+p, g, h]
        w1p = np.empty((128, NG, 2, 128), e4)
        for kt in range(2):
            w1p[:, :, kt, :] = w1c[kt * 128:(kt + 1) * 128]
        w1_cores.append(w1p)

    CW = (W1flat[0] * W2).astype(np.float32)

    def _hilo(w):
        hi = w.astype(ml_dtypes.bfloat16)
        lo = (w - hi.astype(np.float32)).astype(ml_dtypes.bfloat16)
        return hi, lo

    rw_in = np.zeros((128, 4), ml_dtypes.bfloat16)
    rw_in[:, 0], rw_in[:, 1] = _hilo(W2)
    rw_in[:, 2], rw_in[:, 3] = _hilo(CW)
    return shp, w1_cores, rw_in, CW


def _decode_outputs(results):
    """Per-core racc (128, 16g cols) f32 -> GV, GD [(i, b, j, r)]."""
    GV = np.empty((L, B, L, 8), np.float32)
    GD = np.empty((L, B, L, 8), np.float32)
    for k in range(NCORES):
        arr = np.concatenate([np.asarray(results[k]["gvgd0"]),
                              np.asarray(results[k]["gvgd1"])], axis=1)
        # col = g*16 + mb*4 + (0:GVhi 1:GVlo 2:GDhi 3:GDlo); m = mb*128 + p
        a = arr.reshape(128, NG, 4, 4)               # [p, g, mb, q]
        gv = (a[..., 0] + a[..., 1]).transpose(2, 0, 1).reshape(M, NG)
        gd = (a[..., 2] + a[..., 3]).transpose(2, 0, 1).reshape(M, NG)
        # m = i*B + b ; g = jl*8 + r
        gv = gv.reshape(L, B, JPER, 8)
        gd = gd.reshape(L, B, JPER, 8)
        GV[:, :, JPER * k:JPER * (k + 1), :] = gv
        GD[:, :, JPER * k:JPER * (k + 1), :] = gd
    return GV, GD


def _combine(x, GV, GD, CW, muO):
    _, mus = _tables()
    fxo = _force(x).reshape(B, V)
    Csum = float(CW.sum())
    Ftot = np.zeros(B, np.float64)
    for r in range(8):
        gval = GV[:, :, :, r].transpose(0, 2, 1).reshape(V, B)
        gdot = Csum - GD[:, :, :, r].transpose(0, 2, 1).reshape(V, B)
        fxt = fxo[:, mus[r]].T
        Ftot += (gdot + gval * fxt).sum(axis=0)
    F = (Ftot / 8.0).astype(np.float32)
    delta = _computeO(x) - F
    return np.float32(((delta - muO[0]) ** 2).mean())


# ---------------------------------------------------------------------------
# entry point
# ---------------------------------------------------------------------------

def kernel(x, W1, b1, W2, b2, muO):
    x = np.asarray(x, np.float32)
    W1 = np.asarray(W1, np.float32)
    b1 = np.asarray(b1, np.float32)
    W2 = np.asarray(W2, np.float32)
    b2 = np.asarray(b2, np.float32)
    muO = np.asarray(muO, np.float32)

    if np.any(b1 != 0.0) or np.any(b2 != 0.0):
        return _numpy_reference(x, W1, b1, W2, b2, muO)

    shp, w1_cores, rw_in, CW = _prepare_inputs(x, W1, W2)

    nc = _get_program()
    from concourse import bass_utils
    in_maps = [{"shp": shp, "w1p": w1_cores[k], "rw": rw_in}
               for k in range(NCORES)]
    res = bass_utils.run_bass_kernel_spmd(nc, in_maps,
                                          core_ids=list(range(NCORES)))

    GV, GD = _decode_outputs(res.results)
    return _combine(x, GV, GD, CW, muO)


# revision 13
# speedup vs baseline: 1.1228x; 1.0069x over previous
"""Trainium2 Bass kernel for nn_ControlModel_g (phi^4 lattice control-variate loss).

Math reformulation (validated to fp32 accuracy against the jax reference):

  The reference evaluates, for each of 16 signed lattice symmetries t and all
  V=256 torus translations s, the tiny MLP g (256->128->1) on the transformed+
  shifted configs, plus its input-gradient at site (0,0), combined with the
  phi^4 force into F[b]; loss = mean((computeO(x) - F - muO)^2).

  1. Symmetry transforms move from x onto W1 (g(T_{-s} R x) = g_R(T_{-s'} x)
     with spatially-transformed weights), so all shifted inputs derive from x
     alone and the force/gradient corrections become fixed permutations.
  2. With b1 == 0 (always true for this model), tanh oddness makes the 8
     sign=-1 transforms algebraically redundant -> half the compute.
  3. The column translation j folds into 16 rotated weight copies
     (W1JBIG[(a,c), (j,r,h)] = W1_r[a, (c-j)%16, h]); the row translation i
     folds into a small shifted-x matrix SH2[(a,c), (i,b)] = x[b,(a+i)%16,c].
     The device work is then one dense matmul Z = SH2^T @ W1JBIG
     (512 x 16384), tanh, and two h-weighted reductions:
         GV = sum_h W2[h] * tanh(Z),   GD = sum_h (W2*W1[0])[h] * tanh(Z)^2
  4. Sharding: data-parallel over the j columns - core k takes j in {2k,2k+1}
     (2048 of the 16384 output columns). No collectives needed; the final
     O(B*V) combine (force permutations, computeO, loss) is host-side numpy.

Device schedule (v2): inputs land as fp8e4; x is split hi+lo so the only fp8
quantization error left is W1's (loss rel err ~7e-3, tol 2e-2). Each of the
16 per-core column chunks is one pair of DoubleRow matmuls (K=256 folded as
[128, 2]) into PSUM; the Activation engine (the bottleneck at 0.83ns/col)
consumes the chunks as 7 variable-size tanh tiles sized to start early and
finish with a small last tile; DVE squares each tile; the h-reductions run as
free small PE matmuls into two PSUM accumulators shipped by two output DMAs
(first 13 chunks early, last 3 at the end).
"""

import numpy as np
import ml_dtypes

L = 16
Y = 4
KAPPA = 0.25
LAM = 0.5
B = 32
V = L * L          # 256
H = 128
NCORES = 8
JPER = L // NCORES         # j values per core = 2
M = L * B                  # 512 rows (i, b)
NG = 16                    # column chunks per core (g = jl*8 + r)
WARM_MMS = 57              # PE p-state warm-up matmuls

# act tile plan: (slot, n_chunks); slots A/B are [128, 1536] f32 PSUM (3
# banks each); accumulators racc0 (chunks 0..12) / racc1 (13..15) take the
# last 2 banks.
TILES = [(0, 1), (1, 3), (0, 3), (1, 3), (0, 3), (1, 2), (0, 1)]
NG0 = 7                    # chunks in racc0 / first output DMA

# ---------------------------------------------------------------------------
# host-side lattice helpers
# ---------------------------------------------------------------------------

def _force(phi):
    nbr = (np.roll(phi, 1, 1) + np.roll(phi, -1, 1)
           + np.roll(phi, 1, 2) + np.roll(phi, -1, 2))
    return 2.0 * KAPPA * nbr - 2.0 * phi - 4.0 * LAM * phi * (phi * phi - 1.0)


def _computeO(x):
    x0 = x.mean(axis=1)
    x0 = x0 - x0.mean(axis=0, keepdims=True)
    return (x0 * np.roll(x0, -Y, axis=1)).mean(axis=1)


def _spatial_ops():
    ops = []
    for k in range(4):
        ops.append(lambda y, k=k: np.rot90(y, k=k, axes=(0, 1)))
        ops.append(lambda y, k=k: np.flip(np.rot90(y, k=k, axes=(0, 1)), axis=0))
    return ops


def _op_tables():
    """Per spatial op r: inverse site permutation (for W1) and the force
    permutation mu_r[s] = pi_r(rho_r^{-1}(s))."""
    ops = _spatial_ops()
    IDX = np.arange(V).reshape(L, L)
    inv_perms, mus = [], []
    for op in ops:
        pi = op(IDX).reshape(-1)
        inv = np.empty(V, np.int64)
        inv[pi] = np.arange(V)
        inv_perms.append(inv)
        rho = np.empty(V, np.int64)
        opIDX = op(IDX)
        for i in range(L):
            for j in range(L):
                shifted = np.roll(np.roll(opIDX, -i, 0), -j, 1).reshape(V)
                rho[i * L + j] = shifted[inv][0]
        rho_inv = np.empty(V, np.int64)
        rho_inv[rho] = np.arange(V)
        mus.append(pi[rho_inv])
    return inv_perms, mus


_TABLES = None

def _tables():
    global _TABLES
    if _TABLES is None:
        _TABLES = _op_tables()
    return _TABLES


# ---------------------------------------------------------------------------
# device program (built once, cached)
# ---------------------------------------------------------------------------

_PROG = None

def _build_program():
    import concourse.bass as bass
    import concourse.tile as tile
    from concourse import bacc, mybir

    f32 = mybir.dt.float32
    bf16 = mybir.dt.bfloat16
    fp8 = mybir.dt.float8e4
    MUL = mybir.AluOpType.mult
    TANH = mybir.ActivationFunctionType.Tanh
    DR = mybir.MatmulPerfMode.DoubleRow

    nc = bacc.Bacc("TRN2", target_bir_lowering=False, debug=False,
                   num_devices=NCORES)
    # shw[p, kt, 0:512] = SH2hi[kt*128+p, m]; [512:1024] = SH2lo (x hi/lo
    # fp8 pair); [1024+g*128:...] = W1 chunks g0, g1 — one DMA covers
    # everything the first act tile needs.
    shw_d = nc.dram_tensor("shw", (128, 2, 1280), fp8, kind="ExternalInput")
    # w1r[p, g-2, kt, h] = W1JBIG_core[kt*128+p, g*128+h] for g = 2..15
    w1r_d = nc.dram_tensor("w1r", (128, NG - 2, 2, 128), fp8,
                           kind="ExternalInput")
    rw_d = nc.dram_tensor("rw", (128, 4), bf16, kind="ExternalInput")
    out0_d = nc.dram_tensor("gvgd0", (128, NG0 * 16), f32, kind="ExternalOutput")
    out1_d = nc.dram_tensor("gvgd1", (128, (NG - NG0) * 16), f32,
                            kind="ExternalOutput")

    with tile.TileContext(nc) as tc:
        with (
            tc.tile_pool(name="consts", bufs=1) as cpool,
            tc.tile_pool(name="zp", bufs=1, space=bass.MemorySpace.PSUM) as zpool,
            tc.tile_pool(name="rp", bufs=1, space=bass.MemorySpace.PSUM) as rpool,
            tc.tile_pool(name="work", bufs=1) as wpool,
        ):
            # Warm the PE p-state with dependency-free dummy matmuls so the
            # tensor engine is ramped when the first real weights land.
            warm_src = wpool.tile([128, 64], bf16, tag="warm_src")
            nc.gpsimd.memset(warm_src[:], 0.03125)
            # Prime the Act engine's tanh table while DMAs are in flight.
            zt0 = wpool.tile([128, 1], f32, tag="prime_in")
            pr0 = wpool.tile([128, 1], bf16, tag="prime_out")
            nc.gpsimd.memset(zt0[:], 0.0)
            nc.scalar.activation(pr0[:], zt0[:], TANH)

            shw_t = cpool.tile([128, 2, 1280], fp8, tag="shw")
            w1r_t = cpool.tile([128, NG - 2, 2, 128], fp8, tag="w1r")
            rw_t = cpool.tile([128, 4], bf16, tag="rw")
            nc.sync.dma_start(shw_t[:], shw_d[:])
            nc.scalar.dma_start(w1r_t[:, 0:6], w1r_d[:, 0:6])
            nc.sync.dma_start(w1r_t[:, 6:NG - 2], w1r_d[:, 6:NG - 2])
            nc.scalar.dma_start(rw_t[:], rw_d[:])
            shhi = shw_t[:, :, 0:512]
            shlo = shw_t[:, :, 512:1024]

            def w1chunk(g):
                if g < 2:
                    return shw_t[:, :, 1024 + g * 128:1152 + g * 128]
                return w1r_t[:, g - 2]

            slots = [zpool.tile([128, 1536], f32, tag="ztA", name="ztA"),
                     zpool.tile([128, 1536], f32, tag="ztB", name="ztB")]
            racc0 = rpool.tile([128, NG0 * 16], f32, tag="racc0")
            racc1 = rpool.tile([128, (NG - NG0) * 16], f32, tag="racc1")
            rsb0 = wpool.tile([128, NG0 * 16], f32, tag="rsb0")
            rsb1 = wpool.tile([128, (NG - NG0) * 16], f32, tag="rsb1")

            for _ in range(WARM_MMS):
                nc.tensor.matmul(slots[0][0:64, 0:64], warm_src[:, 0:64],
                                 warm_src[:], start=True, stop=True)

            # tile t covers chunks [gbase[t], gbase[t]+ng)
            gbase = []
            acc = 0
            for _, ngc in TILES:
                gbase.append(acc)
                acc += ngc
            ats, asqs = {}, {}

            def emit_z(t):
                slot, ngc = TILES[t]
                zt = slots[slot]
                for gi in range(ngc):
                    g = gbase[t] + gi
                    out = zt[:, gi * 512:(gi + 1) * 512]
                    nc.tensor.matmul(out, w1chunk(g), shhi,
                                     start=True, stop=False, perf_mode=DR)
                    nc.tensor.matmul(out, w1chunk(g), shlo,
                                     start=False, stop=True, perf_mode=DR)

            def emit_act(t):
                slot, ngc = TILES[t]
                at = wpool.tile([128, ngc * 512], bf16, tag=f"at{t}", name="at")
                nc.scalar.activation(at[:], slots[slot][:, 0:ngc * 512], TANH)
                ats[t] = at

            def emit_asq(t):
                _, ngc = TILES[t]
                asq = wpool.tile([128, ngc * 512], bf16, tag=f"asq{t}",
                                 name="asq")
                nc.vector.tensor_tensor(asq[:], ats[t][:], ats[t][:], MUL)
                asqs[t] = asq

            def emit_red(t):
                _, ngc = TILES[t]
                for gi in range(ngc):
                    g = gbase[t] + gi
                    racc, base = (racc0, g * 16) if g < NG0 else \
                                 (racc1, (g - NG0) * 16)
                    for mb in range(4):
                        cs = slice(gi * 512 + mb * 128, gi * 512 + mb * 128 + 128)
                        for which, src in ((0, ats[t]), (1, asqs[t])):
                            off = base + mb * 4 + 2 * which
                            nc.tensor.matmul(racc[:, off:off + 2], src[:, cs],
                                             rw_t[:, 2 * which:2 * which + 2],
                                             start=True, stop=True)

            NT = len(TILES)
            for t in range(NT):
                emit_z(t)
                emit_act(t)
                if t == 4:
                    # racc0 (tiles 0-2) is complete after red(2); emitting the
                    # copy here slots it into the DVE idle window before
                    # asq(4), so the early output DMA departs mid-stream.
                    emit_red(2)
                    nc.vector.tensor_copy(rsb0[:], racc0[:])
                    nc.sync.dma_start(out0_d[:], rsb0[:])
                emit_asq(t)
                if t >= 2 and t != 4:
                    emit_red(t - 2)
            emit_red(NT - 2)
            emit_red(NT - 1)
            nc.vector.tensor_copy(rsb1[:], racc1[:])
            nc.sync.dma_start(out1_d[:], rsb1[:])

    nc.compile()
    return nc


def _get_program():
    global _PROG
    if _PROG is None:
        _PROG = _build_program()
    return _PROG


# ---------------------------------------------------------------------------
# numpy fallback (general b1/b2; never hit for this model's inputs)
# ---------------------------------------------------------------------------

def _numpy_reference(x, W1, b1, W2, b2, muO):
    def transforms(x):
        outs = []
        for sign in (1.0, -1.0):
            sx = sign * x
            for k in range(4):
                rx = np.rot90(sx, k=k, axes=(1, 2))
                outs.append(rx)
                outs.append(np.flip(rx, axis=1))
        return np.stack(outs)

    idx = (np.arange(L)[:, None] + np.arange(L)[None, :]) % L
    Ftot = np.zeros(B, np.float32)
    for tx in transforms(x):
        fx = _force(tx).reshape(B, V)
        sh = tx[:, idx, :][:, :, :, idx]
        shifts = np.transpose(sh, (1, 3, 0, 2, 4)).reshape(V, B, V)
        z = shifts @ W1 + b1
        h = np.tanh(z)
        gvals = h @ W2 + b2[0]
        grads = ((1.0 - h * h) * W2) @ W1[0]
        Ftot += (grads + gvals * fx.T).sum(axis=0)
    F = Ftot / 16.0
    delta = _computeO(x) - F
    return np.float32(((delta - muO[0]) ** 2).mean())


# ---------------------------------------------------------------------------
# host-side input prep / output decode
# ---------------------------------------------------------------------------

def _prepare_inputs(x, W1, W2):
    inv_perms, _ = _tables()
    W1flat = W1.reshape(V, H)
    e4 = ml_dtypes.float8_e4m3

    # SH2[(a,c), (i,b)] = x[b, (a+i)%L, c]; x split hi+lo in fp8
    SH2 = np.empty((V, M), np.float32)
    for i in range(L):
        SH2[:, i * B:(i + 1) * B] = np.roll(x, -i, axis=1).reshape(B, V).T
    shhi = SH2.astype(e4)
    shlo = (SH2 - shhi.astype(np.float32)).astype(e4)

    # W1JBIG columns (jl, r, h); per-core slice j in {2k, 2k+1}
    W1r_imgs = [W1flat[inv].reshape(L, L, H) for inv in inv_perms]
    shw_cores, w1r_cores = [], []
    for k in range(NCORES):
        blk = np.empty((V, JPER, 8, H), np.float32)
        for jl in range(JPER):
            j = JPER * k + jl
            for r in range(8):
                blk[:, jl, r, :] = np.roll(W1r_imgs[r], j, axis=1).reshape(V, H)
        w1c = blk.reshape(V, NG, 128).astype(e4)   # [kt*128+p, g, h]
        shw = np.empty((128, 2, 1280), e4)
        w1r = np.empty((128, NG - 2, 2, 128), e4)
        for kt in range(2):
            rows = slice(kt * 128, (kt + 1) * 128)
            shw[:, kt, 0:512] = shhi[rows]
            shw[:, kt, 512:1024] = shlo[rows]
            shw[:, kt, 1024:1152] = w1c[rows, 0]
            shw[:, kt, 1152:1280] = w1c[rows, 1]
            w1r[:, :, kt, :] = w1c[rows, 2:NG]
        shw_cores.append(shw)
        w1r_cores.append(w1r)

    CW = (W1flat[0] * W2).astype(np.float32)

    def _hilo(w):
        hi = w.astype(ml_dtypes.bfloat16)
        lo = (w - hi.astype(np.float32)).astype(ml_dtypes.bfloat16)
        return hi, lo

    rw_in = np.zeros((128, 4), ml_dtypes.bfloat16)
    rw_in[:, 0], rw_in[:, 1] = _hilo(W2)
    rw_in[:, 2], rw_in[:, 3] = _hilo(CW)
    return shw_cores, w1r_cores, rw_in, CW


def _decode_outputs(results):
    """Per-core racc (128, 16g cols) f32 -> GV, GD [(i, b, j, r)]."""
    GV = np.empty((L, B, L, 8), np.float32)
    GD = np.empty((L, B, L, 8), np.float32)
    for k in range(NCORES):
        arr = np.concatenate([np.asarray(results[k]["gvgd0"]),
                              np.asarray(results[k]["gvgd1"])], axis=1)
        # col = g*16 + mb*4 + (0:GVhi 1:GVlo 2:GDhi 3:GDlo); m = mb*128 + p
        a = arr.reshape(128, NG, 4, 4)               # [p, g, mb, q]
        gv = (a[..., 0] + a[..., 1]).transpose(2, 0, 1).reshape(M, NG)
        gd = (a[..., 2] + a[..., 3]).transpose(2, 0, 1).reshape(M, NG)
        # m = i*B + b ; g = jl*8 + r
        gv = gv.reshape(L, B, JPER, 8)
        gd = gd.reshape(L, B, JPER, 8)
        GV[:, :, JPER * k:JPER * (k + 1), :] = gv
        GD[:, :, JPER * k:JPER * (k + 1), :] = gd
    return GV, GD


def _combine(x, GV, GD, CW, muO):
    _, mus = _tables()
    fxo = _force(x).reshape(B, V)
    Csum = float(CW.sum())
    Ftot = np.zeros(B, np.float64)
    for r in range(8):
        gval = GV[:, :, :, r].transpose(0, 2, 1).reshape(V, B)
        gdot = Csum - GD[:, :, :, r].transpose(0, 2, 1).reshape(V, B)
        fxt = fxo[:, mus[r]].T
        Ftot += (gdot + gval * fxt).sum(axis=0)
    F = (Ftot / 8.0).astype(np.float32)
    delta = _computeO(x) - F
    return np.float32(((delta - muO[0]) ** 2).mean())


# ---------------------------------------------------------------------------
# entry point
# ---------------------------------------------------------------------------

def kernel(x, W1, b1, W2, b2, muO):
    x = np.asarray(x, np.float32)
    W1 = np.asarray(W1, np.float32)
    b1 = np.asarray(b1, np.float32)
    W2 = np.asarray(W2, np.float32)
    b2 = np.asarray(b2, np.float32)
    muO = np.asarray(muO, np.float32)

    if np.any(b1 != 0.0) or np.any(b2 != 0.0):
        return _numpy_reference(x, W1, b1, W2, b2, muO)

    shw_cores, w1r_cores, rw_in, CW = _prepare_inputs(x, W1, W2)

    nc = _get_program()
    from concourse import bass_utils
    in_maps = [{"shw": shw_cores[k], "w1r": w1r_cores[k], "rw": rw_in}
               for k in range(NCORES)]
    res = bass_utils.run_bass_kernel_spmd(nc, in_maps,
                                          core_ids=list(range(NCORES)))

    GV, GD = _decode_outputs(res.results)
    return _combine(x, GV, GD, CW, muO)
